# revision 38
# baseline (speedup 1.0000x reference)
"""Trainium2 Bass kernel for ContrastiveVideoAudioSimilarity.

Math (per batch element b, fully folded form):
  q        = probe @ wq.T + bq                      # [1024] -> heads [16, 64]
  ck[h,:]  = q[h] @ wk[h*64:(h+1)*64, :]            # [16, 1024]  (host folded)
  scores   = x @ ck.T / 8                           # [T*S, 16]; bk shift cancels in softmax
  attn     = softmax over S
  cx[t,h]  = sum_s attn[s,h] * x[t,s,:]             # [T, 16, 1024]
  ctx[t,h*64+d] = cx[t,h] @ wv[h*64+d,:] + bv       # per-head V proj of pooled vector
  pooled   = ctx @ wo.T + bo ; LayerNorm ; proj ; L2-normalize both sides; scaled dot.

Sharding: batch B=8, one batch element per NeuronCore (8 cores), params replicated.

Host precomputes layouts so the device never transposes activations on the PE:
  xn   [128 si, T, 2 so, 1024 c] bf16  natural x (K=s matmuls: attn-weighted sum)
  xt8  [128 ci, T, 8 co, 256 s]  fp8e4 transposed x (K=c matmul: scores)
  audt [128 di, 4 do, 2048 l]    bf16  transposed audio (K=d matmul: similarity)
  ck8  [128 ci, 8 co, 32 m]      fp8e4 folded key probe, scaled x1024 to stay in
                                       fp8 normal range (softmax exp rescales)
The audio per-token 1/||a_l|| is folded into the output column scale instead of
pre-scaling audio rows (norms via ones-matmul over the transposed layout).
"""

import sys

for _p in ("/opt/trn_rl_repo", "/root/.axon_site/_ro/trn_rl_repo"):
    if _p not in sys.path:
        sys.path.insert(0, _p)

import numpy as np
import ml_dtypes

import concourse.bass as bass
import concourse.tile as tile
from concourse import bacc, mybir
from concourse.masks import make_identity

F32 = mybir.dt.float32
BF16 = mybir.dt.bfloat16
FP8 = mybir.dt.float8e4
FP8E3 = mybir.dt.float8e3

B, T, S, DV, DA, NH, DH, L2 = 8, 32, 256, 1024, 512, 16, 64, 2048
EPS = 1e-6
FPG = 4  # frames per group
CKSCALE = 1024.0  # fp8 range lift for ck; undone in the softmax exp


def build_nc(n_groups=T // FPG):
    """Build the per-core Bass program. n_groups*FPG = number of frames."""
    nT = n_groups * FPG
    nc = bacc.Bacc("TRN2", target_bir_lowering=False, debug=False)

    xn = nc.dram_tensor("xn", [128, nT, 2, DV], FP8E3, kind="ExternalInput").ap()
    xt8 = nc.dram_tensor("xt8", [128, nT, 8, S], FP8, kind="ExternalInput").ap()
    audt = nc.dram_tensor("audt", [128, 4, L2], BF16, kind="ExternalInput").ap()
    ck8 = nc.dram_tensor("ck8", [128, 8, 32], FP8, kind="ExternalInput").ap()
    wvt = nc.dram_tensor("wvt", [128, 8, DV], BF16, kind="ExternalInput").ap()
    wot = nc.dram_tensor("wot", [128, 8, DV], BF16, kind="ExternalInput").ap()
    bo2 = nc.dram_tensor("bo2", [1, DV], F32, kind="ExternalInput").ap()
    pjt = nc.dram_tensor("pjt", [128, 8, DA], BF16, kind="ExternalInput").ap()
    pjb = nc.dram_tensor("pjb", [1, DA], F32, kind="ExternalInput").ap()
    sca = nc.dram_tensor("sca", [1, 2], F32, kind="ExternalInput").ap()
    out = nc.dram_tensor("out", [nT, L2], F32, kind="ExternalOutput").ap()

    with tile.TileContext(nc) as tc:
        with (
            tc.tile_pool(name="const", bufs=1) as constp,
            tc.tile_pool(name="persist", bufs=1) as persist,
        ):
            # ---- constants / params resident in SBUF ----
            idb = constp.tile([128, 128], BF16)
            make_identity(nc, idb)
            idf = constp.tile([128, 128], F32)
            make_identity(nc, idf)
            ck_s = constp.tile([128, 8, 32], FP8)
            nc.gpsimd.dma_start(ck_s[:], ck8)
            wvt_s = persist.tile([128, 8, DV], BF16)
            wot_s = persist.tile([128, 8, DV], BF16)
            pjt_s = persist.tile([128, 8, DA], BF16)
            audt_s = persist.tile([128, 4, L2], BF16)
            bo2_s = constp.tile([nT, DV], F32)
            pjb_s = constp.tile([nT, DA], F32)
            sca_s = constp.tile([nT, 2], F32)
            rml16 = persist.tile([16, L2], F32)  # 1/||a_l||, bcast on 16 parts
            ones_s = constp.tile([128, 1], BF16)
            nc.vector.memset(ones_s, 1.0)
            epsb = constp.tile([128, 1], F32)
            nc.vector.memset(epsb, EPS)
            ickb = constp.tile([128, 1], F32)
            nc.vector.memset(ickb, 1.0 / CKSCALE)

            def bcast_dma(t_, src, parts):
                """Load a [1, free] DRAM row broadcast across `parts` partitions."""
                src_b = bass.AP(
                    tensor=src.tensor, offset=src.offset,
                    ap=[[0, parts]] + list(src.ap[1:]),
                )
                nc.gpsimd.dma_start(out=t_[:], in_=src_b)

            # cx^T accumulator: cxt[ci, co, t, h] = cx[t, h, co*128+ci]
            cxt = persist.tile([128, 8, nT, NH], BF16)
            # ctx^T[e, t] (stage-2 output, filled per half)
            ctxt = persist.tile([128, 8, nT], BF16)
            HT = nT // 2  # frames per tail-half

            half_state = {}

            def emit_s2(half, s3, ps234, mmb=2, eos=range(8)):
                """Stage 2: per-head V-projection ctx^T[e, t]."""
                t0 = half * HT
                ts_ = slice(t0, t0 + HT)
                for eo in eos:
                    ctp = ps234.tile([128, HT], F32, tag="mm", bufs=mmb)
                    for hh in range(2):
                        h = 2 * eo + hh
                        for co in range(8):
                            nc.tensor.matmul(
                                ctp[64 * hh:64 * (hh + 1), :],
                                wvt_s[:, co, h * DH:(h + 1) * DH],
                                cxt[:, co, ts_, h],
                                start=(co == 0),
                                stop=(co == 7),
                                skip_group_check=True,
                            )
                    if eo % 2 == 0:
                        nc.vector.tensor_copy(out=ctxt[:, eo, ts_], in_=ctp[:])
                    else:
                        nc.scalar.activation(
                            out=ctxt[:, eo, ts_], in_=ctp[:],
                            func=mybir.ActivationFunctionType.Copy,
                        )

            def emit_s3(half, s3, ps234, mmb=2):
                """Stage 3: wo projection + LayerNorm z-score."""
                t0 = half * HT
                ts_ = slice(t0, t0 + HT)
                pooled = s3.tile([HT, DV], F32, tag=f"pl{half}")
                for n in range(2):
                    pp = ps234.tile([HT, 512], F32, tag="mm", bufs=mmb)
                    for eo in range(8):
                        nc.tensor.matmul(
                            pp[:],
                            ctxt[:, eo, ts_],
                            wot_s[:, eo, 512 * n:512 * (n + 1)],
                            start=(eo == 0),
                            stop=(eo == 7),
                        )
                    nc.vector.tensor_tensor(
                        out=pooled[:, 512 * n:512 * (n + 1)], in0=pp[:],
                        in1=bo2_s[:HT, 512 * n:512 * (n + 1)],
                        op=mybir.AluOpType.add,
                    )
                lst = s3.tile([HT, 2, 6], F32, tag=f"ls{half}")
                nc.vector.bn_stats(out=lst[:, 0, :], in_=pooled[:, 0:512])
                nc.vector.bn_stats(out=lst[:, 1, :], in_=pooled[:, 512:1024])
                lmv = s3.tile([HT, 2], F32, tag=f"lm{half}")
                nc.vector.bn_aggr(out=lmv[:], in_=lst[:])
                sd = s3.tile([HT, 1], F32, tag=f"sd{half}")
                nc.scalar.activation(
                    out=sd[:], in_=lmv[:, 1:2],
                    func=mybir.ActivationFunctionType.Sqrt, bias=epsb[:HT],
                )
                rstd = s3.tile([HT, 1], F32, tag=f"rs{half}")
                nc.vector.reciprocal(out=rstd[:], in_=sd[:])
                # ln_g/ln_b are folded into pjt/pjb host-side; only z-score here
                nc.vector.tensor_scalar(
                    out=pooled[:], in0=pooled[:],
                    scalar1=lmv[:, 0:1], scalar2=rstd[:],
                    op0=mybir.AluOpType.subtract, op1=mybir.AluOpType.mult,
                )
                half_state[(half, "pooled")] = pooled

            def emit_proj(half, s3, ps234, mmb=2, trb=1):
                """Pooled transpose, audio-dim projection, vt norm, vt^T."""
                pooled = half_state[(half, "pooled")]
                plt = s3.tile([128, 8, HT], BF16, tag=f"pt{half}")
                for fo in range(8):
                    ptp = ps234.tile([128, HT], F32, tag="tr", bufs=trb)
                    nc.tensor.transpose(
                        ptp[:], pooled[:, 128 * fo:128 * (fo + 1)], idf[:HT, :HT]
                    )
                    nc.vector.tensor_copy(out=plt[:, fo, :], in_=ptp[:])
                vtp = ps234.tile([HT, DA], F32, tag="mm", bufs=mmb)
                for fo in range(8):
                    nc.tensor.matmul(
                        vtp[:],
                        plt[:, fo, :],
                        pjt_s[:, fo, :],
                        start=(fo == 0),
                        stop=(fo == 7),
                    )
                vt = s3.tile([HT, DA], F32, tag=f"vt{half}")
                nc.vector.tensor_tensor(
                    out=vt[:], in0=vtp[:], in1=pjb_s[:HT, :],
                    op=mybir.AluOpType.add,
                )
                # s_t = exp(logit_scale) / ||vt||
                vst = s3.tile([HT, 6], F32, tag=f"vs{half}")
                nc.vector.bn_stats(out=vst[:], in_=vt[:])
                vmv = s3.tile([HT, 2], F32, tag=f"vm{half}")
                nc.vector.bn_aggr(out=vmv[:], in_=vst[:])
                vss = s3.tile([HT, 1], F32, tag=f"vq{half}")
                nc.vector.tensor_tensor(
                    out=vss[:], in0=vmv[:, 0:1], in1=vmv[:, 0:1],
                    op=mybir.AluOpType.mult,
                )
                nc.vector.tensor_add(vss[:], vss[:], vmv[:, 1:2])
                nc.scalar.activation(
                    out=vss[:], in_=vss[:],
                    func=mybir.ActivationFunctionType.Sqrt, scale=float(DA),
                )
                st = s3.tile([HT, 1], F32, tag=f"st{half}")
                nc.vector.reciprocal(out=st[:], in_=vss[:])
                nc.vector.tensor_scalar_mul(
                    out=st[:], in0=st[:], scalar1=sca_s[:HT, 0:1]
                )
                # vt^T as bf16 for the similarity matmul
                vttb = s3.tile([128, 4, HT], BF16, tag=f"vb{half}")
                for do in range(4):
                    vtp2 = ps234.tile([128, HT], F32, tag="tr", bufs=trb)
                    nc.tensor.transpose(
                        vtp2[:], vt[:, 128 * do:128 * (do + 1)], idf[:HT, :HT]
                    )
                    nc.vector.tensor_copy(out=vttb[:, do, :], in_=vtp2[:])
                half_state[half] = (vttb, st)

            def emit_sim(half, s3, ps234, mmb=2):
                """Stage 4: similarity vs all audio tokens."""
                t0 = half * HT
                ts_ = slice(t0, t0 + HT)
                vttb, st = half_state[half]
                for lc in range(4):
                    smp = ps234.tile([HT, 512], F32, tag="mm", bufs=mmb)
                    for do in range(4):
                        nc.tensor.matmul(
                            smp[:],
                            vttb[:, do, :],
                            audt_s[:, do, 512 * lc:512 * (lc + 1)],
                            start=(do == 0),
                            stop=(do == 3),
                        )
                    # fold 1/||a_l|| (column) then s_t (row) + bias
                    o1 = s3.tile([HT, 512], F32, tag=f"o1{half}", bufs=2)
                    nc.vector.tensor_tensor(
                        out=o1[:], in0=smp[:],
                        in1=rml16[:HT, 512 * lc:512 * (lc + 1)],
                        op=mybir.AluOpType.mult,
                    )
                    nc.vector.tensor_scalar(
                        out=o1[:], in0=o1[:],
                        scalar1=st[:], scalar2=sca_s[:HT, 1:2],
                        op0=mybir.AluOpType.mult, op1=mybir.AluOpType.add,
                    )
                    nc.scalar.dma_start(out=out[ts_, 512 * lc:512 * (lc + 1)], in_=o1[:])

            # ---- stage 1: per-frame attention pooling, tail interleaved ----
            with (
                tc.tile_pool(name="xb", bufs=3) as xbp,
                tc.tile_pool(name="xt", bufs=3) as xtp,
                tc.tile_pool(name="sm", bufs=3) as smp_,
                tc.tile_pool(name="at", bufs=4) as atp_,
                tc.tile_pool(name="audp", bufs=1) as audp,
                tc.tile_pool(name="dram", bufs=1, space="DRAM") as dramp,
                tc.tile_pool(name="s3", bufs=1) as s3,
                tc.tile_pool(name="ps_sc", bufs=2, space="PSUM") as ps_sc,
                tc.tile_pool(name="ps_cx", bufs=1, space="PSUM") as ps_cx,
                tc.tile_pool(name="ps_xt", bufs=1, space="PSUM") as ps_xt,
                tc.tile_pool(name="ps234", bufs=1, space="PSUM") as ps234,
            ):
                def emit_audio_square(a2, dos):
                    """Square the transposed audio (for column norms)."""
                    for do in dos:
                        nc.vector.tensor_tensor(
                            out=a2[:, do, :], in0=audt_s[:, do, :],
                            in1=audt_s[:, do, :], op=mybir.AluOpType.mult,
                        )

                def emit_audio_nrm_mm(nrm_row, a2, lcs):
                    """Column sum-of-squares via ones-matmul (K=d)."""
                    for lc in lcs:
                        nps = ps_xt.tile([1, 512], F32, tag="xt", bufs=1)
                        for do in range(4):
                            nc.tensor.matmul(
                                nps[:],
                                ones_s[:],
                                a2[:, do, 512 * lc:512 * (lc + 1)],
                                start=(do == 0),
                                stop=(do == 3),
                            )
                        nc.vector.tensor_copy(
                            out=nrm_row[:, 512 * lc:512 * (lc + 1)], in_=nps[:]
                        )

                def emit_audio_norm(audp, nrm_row):
                    """rml16[t, l] = 1/||a_l|| broadcast over 16 partitions.

                    The sqrt+reciprocal run in a [128, 16] spread so all
                    DVE/ACT lanes are active (a [16, 2048] reciprocal is
                    ~13us on DVE).
                    """
                    # reshape [1, 2048] -> [128, 16] via DRAM bounce (DRAM APs
                    # are linear; SBUF partition dims can't alias free dims)
                    nrm_d = dramp.tile([1, L2], F32, tag="nd")
                    nc.gpsimd.dma_start(out=nrm_d[:], in_=nrm_row[:])
                    n128 = audp.tile([128, 16], F32, tag="n128")
                    nc.gpsimd.dma_start(
                        out=n128[:],
                        in_=nrm_d[:].rearrange("o (p j) -> (o p) j", p=128),
                    )
                    nc.scalar.activation(
                        out=n128[:], in_=n128[:],
                        func=mybir.ActivationFunctionType.Sqrt,
                    )
                    r128 = audp.tile([128, 16], F32, tag="r128")
                    nc.vector.reciprocal(out=r128[:], in_=n128[:])
                    rml_d = dramp.tile([1, L2], F32, tag="rd")
                    nc.gpsimd.dma_start(
                        out=rml_d[:].rearrange("o (p j) -> (o p) j", p=128),
                        in_=r128[:],
                    )
                    rml_b = bass.AP(
                        tensor=rml_d[:].tensor, offset=rml_d[:].offset,
                        ap=[[0, 16], [1, L2]],
                    )
                    nc.gpsimd.dma_start(out=rml16[:], in_=rml_b)

                def emit_attn_cx(g, xbg, attn_s):
                    """cx for group g (one group late so PE isn't
                    head-of-line blocked on g's softmax)."""
                    # cx^T[c, h] = sum_s xb[s, c] * attn[s, h]
                    for f in range(FPG):
                        cxp = ps_cx.tile([128, 128], F32, tag="cx", bufs=2)
                        for co in range(8):
                            for so in range(2):
                                nc.tensor.matmul(
                                    cxp[:, co * NH:(co + 1) * NH],
                                    xbg[:, f, so, co * 128:(co + 1) * 128],
                                    attn_s[:, f, so, :],
                                    start=(so == 0),
                                    stop=(so == 1),
                                )
                        t = g * FPG + f
                        nc.vector.tensor_copy(
                            out=cxt[:, :, t, :],
                            in_=cxp.rearrange("p (co h) -> p co h", co=8),
                        )

                pend = None
                a2 = audp.tile([128, 4, L2], BF16, tag="a2")
                nrm_row = audp.tile([1, L2], F32, tag="nr")
                for g in range(n_groups):
                    if g == 1:
                        nc.gpsimd.dma_start(out=audt_s[:], in_=audt)
                    if g == 2:
                        emit_audio_square(a2, (0, 1))
                    if g == 3:
                        emit_audio_square(a2, (2, 3))
                    if g == 4:
                        nc.gpsimd.dma_start(wvt_s[:], wvt)
                        emit_audio_nrm_mm(nrm_row, a2, (0, 1))
                    if g == 5:
                        nc.gpsimd.dma_start(wot_s[:], wot)
                        bcast_dma(bo2_s, bo2, nT)
                        emit_audio_nrm_mm(nrm_row, a2, (2, 3))
                        emit_audio_norm(audp, nrm_row)
                    if g == 6:
                        nc.gpsimd.dma_start(pjt_s[:], pjt)
                        bcast_dma(pjb_s, pjb, nT)
                        bcast_dma(sca_s, sca, nT)
                    # group DMAs: transposed x (fp8, scores) + natural x (bf16, cx)
                    xtg = xtp.tile([128, FPG, 8, S], FP8)  # [ci, f, co, s]
                    nc.gpsimd.dma_start(
                        out=xtg[:], in_=xt8[:, g * FPG:(g + 1) * FPG]
                    )
                    xbg = xbp.tile([128, FPG, 2, DV], FP8E3)  # [si, f, so, c]
                    nc.gpsimd.dma_start(
                        out=xbg[:], in_=xn[:, g * FPG:(g + 1) * FPG]
                    )
                    # scores^T: [16@32f, s=256] per frame, packed on partitions.
                    # ck is zero-padded to M=32 so every partition row of scp is
                    # written (junk-but-finite rows 16..32 of each block).
                    scp = ps_sc.tile([128, S], F32)
                    for f in range(FPG):
                        for co in range(8):
                            nc.tensor.matmul(
                                scp[32 * f:32 * f + 32, :],
                                ck_s[:, co, :],
                                xtg[:, f, co, :],
                                start=(co == 0),
                                stop=(co == 7),
                                tile_position=(0, 32 * f),
                            )
                    # softmax over s (free dim), whole group at once; scores are
                    # scaled by CKSCALE so the exp rescales by 1/CKSCALE
                    et = smp_.tile([128, S], F32)
                    esum = smp_.tile([128, 1], F32)
                    rsum = smp_.tile([128, 1], F32)
                    attn_t = atp_.tile([128, S], BF16)  # attn^T [h@32f, s]
                    # scores/CKSCALE are bounded (|s|<~0.4) so exp cannot
                    # overflow: skip the max-subtraction entirely
                    nc.scalar.activation(
                        out=et[:], in_=scp[:],
                        func=mybir.ActivationFunctionType.Exp,
                        scale=1.0 / CKSCALE, accum_out=esum[:],
                    )
                    nc.vector.reciprocal(out=rsum[:], in_=esum[:])
                    nc.vector.tensor_scalar_mul(attn_t[:], et[:], rsum[:])
                    # attn^T via XBAR DMA transpose on the idle SP HWDGE ring:
                    # no PE<->DVE ping-pong, and a full group-period of slack
                    # before cx consumes it
                    attn_s = atp_.tile([128, FPG, 2, NH], BF16, tag="as")
                    for f in range(FPG):
                        for so in range(2):
                            nc.sync.dma_start(
                                out=attn_s[:, f, so, :],
                                in_=attn_t[32 * f:32 * f + 16,
                                           128 * so:128 * (so + 1)],
                                transpose=True,
                            )
                    # previous group's cx (keeps PE fed while this group's
                    # softmax runs on vector/scalar)
                    if pend is not None:
                        emit_attn_cx(*pend)
                    pend = (g, xbg, attn_s)
                    # a sliver of tail half-0 covers the last softmax's
                    # vector latency without pushing cx(7) far down the FIFO
                    if g == 7:
                        emit_s2(0, s3, ps234, eos=range(0, 3))
                emit_attn_cx(*pend)
                # tail: half-1 phases interleaved so PE never head-of-line
                # blocks on half-0's vector chains (sim-0 slots the gaps)
                emit_s2(0, s3, ps234, eos=range(3, 8))
                emit_s3(0, s3, ps234)
                emit_s2(1, s3, ps234, eos=range(0, 4))
                emit_proj(0, s3, ps234)
                emit_s2(1, s3, ps234, eos=range(4, 8))
                emit_sim(0, s3, ps234)
                emit_s3(1, s3, ps234)
                emit_proj(1, s3, ps234)
                emit_sim(1, s3, ps234)

    nc.compile()
    return nc


def host_fold(probe, wq, wk, bq, wv, bv, wo, bo, ln_g, ln_b, proj_w, proj_b,
              logit_scale, logit_bias):
    """Fold weights on the host into device-friendly layouts."""
    f64 = np.float64
    qvec = probe.reshape(-1).astype(f64) @ wq.astype(f64).T + bq.astype(f64)
    q = qvec.reshape(NH, DH)
    ck = np.stack(
        [q[h] @ wk.astype(f64)[h * DH:(h + 1) * DH, :] for h in range(NH)]
    )  # [16, 1024]
    ck /= np.sqrt(f64(DH))
    # zero-pad heads to M=32 so the scores matmul writes full 32-row blocks;
    # scale into fp8 normal range (values ~1e-3 are subnormal in e4m3)
    ckp = np.zeros((32, DV), np.float64)
    ckp[:NH] = ck * CKSCALE
    # ck8[ci, co, m] = ckp[m, co*128+ci]
    ck8 = np.ascontiguousarray(
        ckp.T.reshape(8, 128, 32).transpose(1, 0, 2)
    ).astype(ml_dtypes.float8_e4m3)

    wvt = np.ascontiguousarray(
        wv.T.reshape(8, 128, DV).transpose(1, 0, 2)).astype(ml_dtypes.bfloat16)
    wot = np.ascontiguousarray(
        wo.T.reshape(8, 128, DV).transpose(1, 0, 2)).astype(ml_dtypes.bfloat16)
    # fold LayerNorm gain/bias into the audio projection:
    #   proj(LN_affine(z)) = z @ (proj_w * g)^T + (proj_b + proj_w @ b)
    pw = proj_w.astype(f64) * ln_g.astype(f64)[None, :]
    pb = proj_b.astype(f64) + proj_w.astype(f64) @ ln_b.astype(f64)
    pjt = np.ascontiguousarray(
        pw.T.reshape(8, 128, DA).transpose(1, 0, 2)).astype(ml_dtypes.bfloat16)
    bo2f = bo.astype(f64) + wo.astype(f64) @ bv.astype(f64)
    sca = np.array([[np.exp(np.float64(logit_scale[0])), logit_bias[0]]],
                   np.float32)
    return dict(
        ck8=ck8, wvt=wvt, wot=wot,
        bo2=bo2f.reshape(1, DV).astype(np.float32),
        pjt=pjt, pjb=pb.reshape(1, DA).astype(np.float32),
        sca=sca,
    )


def host_layouts(video_b, audio_b):
    """Per-core activation layouts (cast + permute only, no FLOPs)."""
    xn = np.ascontiguousarray(
        video_b.reshape(T, 2, 128, DV).transpose(2, 0, 1, 3)
    ).astype(ml_dtypes.float8_e3m4)
    xt8 = np.ascontiguousarray(
        video_b.reshape(T, S, 8, 128).transpose(3, 0, 2, 1)
    ).astype(ml_dtypes.float8_e4m3)
    audt = np.ascontiguousarray(
        audio_b.T.reshape(4, 128, L2).transpose(1, 0, 2)
    ).astype(ml_dtypes.bfloat16)
    return dict(xn=xn, xt8=xt8, audt=audt)


_NC_CACHE = {}


def kernel(video_x, audio_x, probe, wq, wk, wv, bq, bk, bv, wo, bo,
           ln_g, ln_b, proj_w, proj_b, logit_scale, logit_bias, T=None, H=None,
           W=None, **_unused):
    from concourse.bass_utils import run_bass_kernel_spmd

    video_x = np.asarray(video_x, np.float32)
    audio_x = np.asarray(audio_x, np.float32)
    params = host_fold(
        np.asarray(probe, np.float32), np.asarray(wq, np.float32),
        np.asarray(wk, np.float32), np.asarray(bq, np.float32),
        np.asarray(wv, np.float32), np.asarray(bv, np.float32),
        np.asarray(wo, np.float32), np.asarray(bo, np.float32),
        np.asarray(ln_g, np.float32), np.asarray(ln_b, np.float32),
        np.asarray(proj_w, np.float32), np.asarray(proj_b, np.float32),
        np.asarray(logit_scale, np.float32), np.asarray(logit_bias, np.float32),
    )
    if "nc" not in _NC_CACHE:
        _NC_CACHE["nc"] = build_nc()
    nc = _NC_CACHE["nc"]
    in_maps = []
    for b in range(B):
        m = dict(params)
        m.update(host_layouts(video_x[b], audio_x[b]))
        in_maps.append(m)
    res = run_bass_kernel_spmd(nc, in_maps, core_ids=list(range(B)), trace=False)
    return np.stack([res.results[b]["out"] for b in range(B)], axis=0)


# revision 39
# speedup vs baseline: 1.5586x; 1.5586x over previous
"""Trainium2 Bass kernel for ContrastiveVideoAudioSimilarity.

Math (per batch element b, fully folded form):
  q        = probe @ wq.T + bq                      # [1024] -> heads [16, 64]
  ck[h,:]  = q[h] @ wk[h*64:(h+1)*64, :]            # [16, 1024]  (host folded)
  scores   = x @ ck.T / 8                           # [T*S, 16]; bk shift cancels in softmax
  attn     = softmax over S
  cx[t,h]  = sum_s attn[s,h] * x[t,s,:]             # [T, 16, 1024]
  ctx[t,h*64+d] = cx[t,h] @ wv[h*64+d,:] + bv       # per-head V proj of pooled vector
  pooled   = ctx @ wo.T + bo ; LayerNorm ; proj ; L2-normalize both sides; scaled dot.

Sharding: batch B=8, one batch element per NeuronCore (8 cores), params replicated.

Host precomputes layouts so the device never transposes activations on the PE:
  xn   [128 si, T, 2 so, 1024 c] bf16  natural x (K=s matmuls: attn-weighted sum)
  xt8  [128 ci, T, 8 co, 256 s]  fp8e4 transposed x (K=c matmul: scores)
  audt [128 di, 4 do, 2048 l]    bf16  transposed audio (K=d matmul: similarity)
  ck8  [128 ci, 8 co, 32 m]      fp8e4 folded key probe, scaled x1024 to stay in
                                       fp8 normal range (softmax exp rescales)
The audio per-token 1/||a_l|| is folded into the output column scale instead of
pre-scaling audio rows (norms via ones-matmul over the transposed layout).
"""

import sys

for _p in ("/opt/trn_rl_repo", "/root/.axon_site/_ro/trn_rl_repo"):
    if _p not in sys.path:
        sys.path.insert(0, _p)

import numpy as np
import ml_dtypes

import concourse.bass as bass
import concourse.tile as tile
from concourse import bacc, mybir
from concourse.masks import make_identity

F32 = mybir.dt.float32
BF16 = mybir.dt.bfloat16
FP8 = mybir.dt.float8e4
FP8E3 = mybir.dt.float8e3

B, T, S, DV, DA, NH, DH, L2 = 8, 32, 256, 1024, 512, 16, 64, 2048
EPS = 1e-6
FPG = 4  # frames per group
CKSCALE = 1024.0  # fp8 range lift for ck; undone in the softmax exp


def build_nc(n_groups=T // FPG):
    """Build the per-core Bass program. n_groups*FPG = number of frames."""
    nT = n_groups * FPG
    nc = bacc.Bacc("TRN2", target_bir_lowering=False, debug=False)

    xn = nc.dram_tensor("xn", [128, nT, 2, DV], FP8E3, kind="ExternalInput").ap()
    xt8 = nc.dram_tensor("xt8", [128, nT, 8, S], FP8, kind="ExternalInput").ap()
    audt = nc.dram_tensor("audt", [128, 4, L2], BF16, kind="ExternalInput").ap()
    ck8 = nc.dram_tensor("ck8", [128, 8, 32], FP8, kind="ExternalInput").ap()
    wvt = nc.dram_tensor("wvt", [128, 8, DV], BF16, kind="ExternalInput").ap()
    wot = nc.dram_tensor("wot", [128, 8, DV], BF16, kind="ExternalInput").ap()
    bo2 = nc.dram_tensor("bo2", [1, DV], F32, kind="ExternalInput").ap()
    pjt = nc.dram_tensor("pjt", [128, 8, DA], BF16, kind="ExternalInput").ap()
    pjb = nc.dram_tensor("pjb", [1, DA], F32, kind="ExternalInput").ap()
    sca = nc.dram_tensor("sca", [1, 2], F32, kind="ExternalInput").ap()
    out = nc.dram_tensor("out", [nT, L2], F32, kind="ExternalOutput").ap()

    with tile.TileContext(nc) as tc:
        with (
            tc.tile_pool(name="const", bufs=1) as constp,
            tc.tile_pool(name="persist", bufs=1) as persist,
        ):
            # ---- constants / params resident in SBUF ----
            idb = constp.tile([128, 128], BF16)
            make_identity(nc, idb)
            idf = constp.tile([128, 128], F32)
            make_identity(nc, idf)
            ck_s = constp.tile([128, 8, 32], FP8)
            nc.gpsimd.dma_start(ck_s[:], ck8)
            wvt_s = persist.tile([128, 8, DV], BF16)
            wot_s = persist.tile([128, 8, DV], BF16)
            pjt_s = persist.tile([128, 8, DA], BF16)
            audt_s = persist.tile([128, 4, L2], BF16)
            bo2_s = constp.tile([nT, DV], F32)
            pjb_s = constp.tile([nT, DA], F32)
            sca_s = constp.tile([nT, 2], F32)
            rml16 = persist.tile([16, L2], F32)  # 1/||a_l||, bcast on 16 parts
            ones_s = constp.tile([128, 1], BF16)
            nc.vector.memset(ones_s, 1.0)
            epsb = constp.tile([128, 1], F32)
            nc.vector.memset(epsb, EPS)
            ickb = constp.tile([128, 1], F32)
            nc.vector.memset(ickb, 1.0 / CKSCALE)

            def bcast_dma(t_, src, parts):
                """Load a [1, free] DRAM row broadcast across `parts` partitions."""
                src_b = bass.AP(
                    tensor=src.tensor, offset=src.offset,
                    ap=[[0, parts]] + list(src.ap[1:]),
                )
                nc.gpsimd.dma_start(out=t_[:], in_=src_b)

            # cx^T accumulator: cxt[ci, co, t, h] = cx[t, h, co*128+ci]
            cxt = persist.tile([128, 8, nT, NH], BF16)
            # ctx^T[e, t] (stage-2 output, filled per half)
            ctxt = persist.tile([128, 8, nT], BF16)
            HT = nT // 2  # frames per tail-half

            half_state = {}

            def emit_s2(half, s3, ps234, mmb=2, eos=range(8)):
                """Stage 2: per-head V-projection ctx^T[e, t]."""
                t0 = half * HT
                ts_ = slice(t0, t0 + HT)
                for eo in eos:
                    ctp = ps234.tile([128, HT], F32, tag="mm", bufs=mmb)
                    for hh in range(2):
                        h = 2 * eo + hh
                        for co in range(8):
                            nc.tensor.matmul(
                                ctp[64 * hh:64 * (hh + 1), :],
                                wvt_s[:, co, h * DH:(h + 1) * DH],
                                cxt[:, co, ts_, h],
                                start=(co == 0),
                                stop=(co == 7),
                                skip_group_check=True,
                            )
                    if eo % 2 == 0:
                        nc.vector.tensor_copy(out=ctxt[:, eo, ts_], in_=ctp[:])
                    else:
                        nc.scalar.activation(
                            out=ctxt[:, eo, ts_], in_=ctp[:],
                            func=mybir.ActivationFunctionType.Copy,
                        )

            def emit_s3(half, s3, ps234, mmb=2):
                """Stage 3: wo projection + LayerNorm z-score."""
                t0 = half * HT
                ts_ = slice(t0, t0 + HT)
                pooled = s3.tile([HT, DV], F32, tag=f"pl{half}")
                for n in range(2):
                    pp = ps234.tile([HT, 512], F32, tag="mm", bufs=mmb)
                    for eo in range(8):
                        nc.tensor.matmul(
                            pp[:],
                            ctxt[:, eo, ts_],
                            wot_s[:, eo, 512 * n:512 * (n + 1)],
                            start=(eo == 0),
                            stop=(eo == 7),
                        )
                    nc.vector.tensor_tensor(
                        out=pooled[:, 512 * n:512 * (n + 1)], in0=pp[:],
                        in1=bo2_s[:HT, 512 * n:512 * (n + 1)],
                        op=mybir.AluOpType.add,
                    )
                lst = s3.tile([HT, 2, 6], F32, tag=f"ls{half}")
                nc.vector.bn_stats(out=lst[:, 0, :], in_=pooled[:, 0:512])
                nc.vector.bn_stats(out=lst[:, 1, :], in_=pooled[:, 512:1024])
                lmv = s3.tile([HT, 2], F32, tag=f"lm{half}")
                nc.vector.bn_aggr(out=lmv[:], in_=lst[:])
                sd = s3.tile([HT, 1], F32, tag=f"sd{half}")
                nc.scalar.activation(
                    out=sd[:], in_=lmv[:, 1:2],
                    func=mybir.ActivationFunctionType.Sqrt, bias=epsb[:HT],
                )
                rstd = s3.tile([HT, 1], F32, tag=f"rs{half}")
                nc.vector.reciprocal(out=rstd[:], in_=sd[:])
                # ln_g/ln_b are folded into pjt/pjb host-side; only z-score here
                nc.vector.tensor_scalar(
                    out=pooled[:], in0=pooled[:],
                    scalar1=lmv[:, 0:1], scalar2=rstd[:],
                    op0=mybir.AluOpType.subtract, op1=mybir.AluOpType.mult,
                )
                half_state[(half, "pooled")] = pooled

            def emit_proj(half, s3, ps234, mmb=2, trb=1):
                """Pooled transpose, audio-dim projection, vt norm, vt^T."""
                pooled = half_state[(half, "pooled")]
                plt = s3.tile([128, 8, HT], BF16, tag=f"pt{half}")
                for fo in range(8):
                    ptp = ps234.tile([128, HT], F32, tag="tr", bufs=trb)
                    nc.tensor.transpose(
                        ptp[:], pooled[:, 128 * fo:128 * (fo + 1)], idf[:HT, :HT]
                    )
                    nc.vector.tensor_copy(out=plt[:, fo, :], in_=ptp[:])
                vtp = ps234.tile([HT, DA], F32, tag="mm", bufs=mmb)
                for fo in range(8):
                    nc.tensor.matmul(
                        vtp[:],
                        plt[:, fo, :],
                        pjt_s[:, fo, :],
                        start=(fo == 0),
                        stop=(fo == 7),
                    )
                vt = s3.tile([HT, DA], F32, tag=f"vt{half}")
                nc.vector.tensor_tensor(
                    out=vt[:], in0=vtp[:], in1=pjb_s[:HT, :],
                    op=mybir.AluOpType.add,
                )
                # s_t = exp(logit_scale) / ||vt||
                vst = s3.tile([HT, 6], F32, tag=f"vs{half}")
                nc.vector.bn_stats(out=vst[:], in_=vt[:])
                vmv = s3.tile([HT, 2], F32, tag=f"vm{half}")
                nc.vector.bn_aggr(out=vmv[:], in_=vst[:])
                vss = s3.tile([HT, 1], F32, tag=f"vq{half}")
                nc.vector.tensor_tensor(
                    out=vss[:], in0=vmv[:, 0:1], in1=vmv[:, 0:1],
                    op=mybir.AluOpType.mult,
                )
                nc.vector.tensor_add(vss[:], vss[:], vmv[:, 1:2])
                nc.scalar.activation(
                    out=vss[:], in_=vss[:],
                    func=mybir.ActivationFunctionType.Sqrt, scale=float(DA),
                )
                st = s3.tile([HT, 1], F32, tag=f"st{half}")
                nc.vector.reciprocal(out=st[:], in_=vss[:])
                nc.vector.tensor_scalar_mul(
                    out=st[:], in0=st[:], scalar1=sca_s[:HT, 0:1]
                )
                # vt^T as bf16 for the similarity matmul
                vttb = s3.tile([128, 4, HT], BF16, tag=f"vb{half}")
                for do in range(4):
                    vtp2 = ps234.tile([128, HT], F32, tag="tr", bufs=trb)
                    nc.tensor.transpose(
                        vtp2[:], vt[:, 128 * do:128 * (do + 1)], idf[:HT, :HT]
                    )
                    nc.vector.tensor_copy(out=vttb[:, do, :], in_=vtp2[:])
                half_state[half] = (vttb, st)

            def emit_sim(half, s3, ps234, mmb=2):
                """Stage 4: similarity vs all audio tokens."""
                t0 = half * HT
                ts_ = slice(t0, t0 + HT)
                vttb, st = half_state[half]
                for lc in range(4):
                    smp = ps234.tile([HT, 512], F32, tag="mm", bufs=mmb)
                    for do in range(4):
                        nc.tensor.matmul(
                            smp[:],
                            vttb[:, do, :],
                            audt_s[:, do, 512 * lc:512 * (lc + 1)],
                            start=(do == 0),
                            stop=(do == 3),
                        )
                    # fold 1/||a_l|| (column) then s_t (row) + bias
                    o1 = s3.tile([HT, 512], F32, tag=f"o1{half}", bufs=2)
                    nc.vector.tensor_tensor(
                        out=o1[:], in0=smp[:],
                        in1=rml16[:HT, 512 * lc:512 * (lc + 1)],
                        op=mybir.AluOpType.mult,
                    )
                    nc.vector.tensor_scalar(
                        out=o1[:], in0=o1[:],
                        scalar1=st[:], scalar2=sca_s[:HT, 1:2],
                        op0=mybir.AluOpType.mult, op1=mybir.AluOpType.add,
                    )
                    nc.scalar.dma_start(out=out[ts_, 512 * lc:512 * (lc + 1)], in_=o1[:])

            # ---- stage 1: per-frame attention pooling, tail interleaved ----
            with (
                tc.tile_pool(name="xb", bufs=3) as xbp,
                tc.tile_pool(name="xt", bufs=3) as xtp,
                tc.tile_pool(name="sm", bufs=3) as smp_,
                tc.tile_pool(name="at", bufs=4) as atp_,
                tc.tile_pool(name="audp", bufs=1) as audp,
                tc.tile_pool(name="dram", bufs=1, space="DRAM") as dramp,
                tc.tile_pool(name="s3", bufs=1) as s3,
                tc.tile_pool(name="ps_sc", bufs=2, space="PSUM") as ps_sc,
                tc.tile_pool(name="ps_cx", bufs=1, space="PSUM") as ps_cx,
                tc.tile_pool(name="ps_xt", bufs=1, space="PSUM") as ps_xt,
                tc.tile_pool(name="ps234", bufs=1, space="PSUM") as ps234,
            ):
                def emit_audio_square(a2, dos):
                    """Square the transposed audio (for column norms)."""
                    for do in dos:
                        nc.vector.tensor_tensor(
                            out=a2[:, do, :], in0=audt_s[:, do, :],
                            in1=audt_s[:, do, :], op=mybir.AluOpType.mult,
                        )

                def emit_audio_nrm_mm(nrm_row, a2, lcs):
                    """Column sum-of-squares via ones-matmul (K=d)."""
                    for lc in lcs:
                        nps = ps_xt.tile([1, 512], F32, tag="xt", bufs=2)
                        for do in range(4):
                            nc.tensor.matmul(
                                nps[:],
                                ones_s[:],
                                a2[:, do, 512 * lc:512 * (lc + 1)],
                                start=(do == 0),
                                stop=(do == 3),
                            )
                        nc.vector.tensor_copy(
                            out=nrm_row[:, 512 * lc:512 * (lc + 1)], in_=nps[:]
                        )

                def emit_audio_norm(audp, nrm_row):
                    """rml16[t, l] = 1/||a_l|| broadcast over 16 partitions.

                    The sqrt+reciprocal run in a [128, 16] spread so all
                    DVE/ACT lanes are active (a [16, 2048] reciprocal is
                    ~13us on DVE).
                    """
                    # reshape [1, 2048] -> [128, 16] via DRAM bounce (DRAM APs
                    # are linear; SBUF partition dims can't alias free dims)
                    nrm_d = dramp.tile([1, L2], F32, tag="nd")
                    nc.gpsimd.dma_start(out=nrm_d[:], in_=nrm_row[:])
                    n128 = audp.tile([128, 16], F32, tag="n128")
                    nc.gpsimd.dma_start(
                        out=n128[:],
                        in_=nrm_d[:].rearrange("o (p j) -> (o p) j", p=128),
                    )
                    nc.scalar.activation(
                        out=n128[:], in_=n128[:],
                        func=mybir.ActivationFunctionType.Sqrt,
                    )
                    r128 = audp.tile([128, 16], F32, tag="r128")
                    nc.vector.reciprocal(out=r128[:], in_=n128[:])
                    rml_d = dramp.tile([1, L2], F32, tag="rd")
                    nc.gpsimd.dma_start(
                        out=rml_d[:].rearrange("o (p j) -> (o p) j", p=128),
                        in_=r128[:],
                    )
                    rml_b = bass.AP(
                        tensor=rml_d[:].tensor, offset=rml_d[:].offset,
                        ap=[[0, 16], [1, L2]],
                    )
                    nc.gpsimd.dma_start(out=rml16[:], in_=rml_b)

                def emit_attn_cx(g, xbg, attn_t):
                    """Attn transpose + cx for group g (one group late so PE
                    isn't head-of-line blocked on g's softmax."""
                    attn_s = atp_.tile([128, FPG, 2, NH], BF16)
                    for f in range(FPG):
                        for so in range(2):
                            atp = ps_xt.tile([128, NH], BF16, tag="xt", bufs=2)
                            nc.tensor.transpose(
                                atp[:],
                                attn_t[32 * f:32 * f + 16, 128 * so:128 * (so + 1)],
                                idb[32 * f:32 * f + 16, 32 * f:32 * f + 16],
                                tile_position=(32 * f, 0),
                            )
                            nc.vector.tensor_copy(out=attn_s[:, f, so, :], in_=atp[:])
                    # cx^T[c, h] = sum_s xb[s, c] * attn[s, h]
                    for f in range(FPG):
                        cxp = ps_cx.tile([128, 128], F32, tag="cx")
                        for co in range(8):
                            for so in range(2):
                                nc.tensor.matmul(
                                    cxp[:, co * NH:(co + 1) * NH],
                                    xbg[:, f, so, co * 128:(co + 1) * 128],
                                    attn_s[:, f, so, :],
                                    start=(so == 0),
                                    stop=(so == 1),
                                )
                        t = g * FPG + f
                        nc.vector.tensor_copy(
                            out=cxt[:, :, t, :],
                            in_=cxp.rearrange("p (co h) -> p co h", co=8),
                        )

                pend = None
                a2 = audp.tile([128, 4, L2], BF16, tag="a2")
                nrm_row = audp.tile([1, L2], F32, tag="nr")
                for g in range(n_groups):
                    if g == 1:
                        nc.gpsimd.dma_start(out=audt_s[:], in_=audt)
                    if g == 2:
                        emit_audio_square(a2, (0, 1))
                    if g == 3:
                        emit_audio_square(a2, (2, 3))
                    if g == 4:
                        nc.gpsimd.dma_start(wvt_s[:], wvt)
                        emit_audio_nrm_mm(nrm_row, a2, (0, 1))
                    if g == 5:
                        nc.gpsimd.dma_start(wot_s[:], wot)
                        bcast_dma(bo2_s, bo2, nT)
                        emit_audio_nrm_mm(nrm_row, a2, (2, 3))
                        emit_audio_norm(audp, nrm_row)
                    if g == 6:
                        nc.gpsimd.dma_start(pjt_s[:], pjt)
                        bcast_dma(pjb_s, pjb, nT)
                        bcast_dma(sca_s, sca, nT)
                    # group DMAs: transposed x (fp8, scores) + natural x (bf16, cx)
                    xtg = xtp.tile([128, FPG, 8, S], FP8)  # [ci, f, co, s]
                    nc.gpsimd.dma_start(
                        out=xtg[:], in_=xt8[:, g * FPG:(g + 1) * FPG]
                    )
                    xbg = xbp.tile([128, FPG, 2, DV], FP8E3)  # [si, f, so, c]
                    nc.gpsimd.dma_start(
                        out=xbg[:], in_=xn[:, g * FPG:(g + 1) * FPG]
                    )
                    # scores^T: [16@32f, s=256] per frame, packed on partitions.
                    # ck is zero-padded to M=32 so every partition row of scp is
                    # written (junk-but-finite rows 16..32 of each block).
                    scp = ps_sc.tile([128, S], F32)
                    for f in range(FPG):
                        for co in range(8):
                            nc.tensor.matmul(
                                scp[32 * f:32 * f + 32, :],
                                ck_s[:, co, :],
                                xtg[:, f, co, :],
                                start=(co == 0),
                                stop=(co == 7),
                                tile_position=(0, 32 * f),
                            )
                    # softmax over s (free dim), whole group at once; scores are
                    # scaled by CKSCALE so the exp rescales by 1/CKSCALE
                    et = smp_.tile([128, S], F32)
                    esum = smp_.tile([128, 1], F32)
                    rsum = smp_.tile([128, 1], F32)
                    attn_t = atp_.tile([128, S], BF16)  # attn^T [h@32f, s]
                    # scores/CKSCALE are bounded (|s|<~0.4) so exp cannot
                    # overflow: skip the max-subtraction entirely
                    nc.scalar.activation(
                        out=et[:], in_=scp[:],
                        func=mybir.ActivationFunctionType.Exp,
                        scale=1.0 / CKSCALE, accum_out=esum[:],
                    )
                    nc.vector.reciprocal(out=rsum[:], in_=esum[:])
                    nc.vector.tensor_scalar_mul(attn_t[:], et[:], rsum[:])
                    # previous group's attn transpose + cx (keeps PE fed while
                    # this group's softmax runs on vector/scalar)
                    if pend is not None:
                        emit_attn_cx(*pend)
                    pend = (g, xbg, attn_t)
                    # a sliver of tail half-0 covers the last softmax's
                    # vector latency without pushing cx(7) far down the FIFO
                    if g == 7:
                        emit_s2(0, s3, ps234, eos=range(0, 3))
                emit_attn_cx(*pend)
                # tail: half-1 phases interleaved so PE never head-of-line
                # blocks on half-0's vector chains (sim-0 slots the gaps)
                emit_s2(0, s3, ps234, eos=range(3, 8))
                emit_s3(0, s3, ps234)
                emit_s2(1, s3, ps234, eos=range(0, 4))
                emit_proj(0, s3, ps234)
                emit_s2(1, s3, ps234, eos=range(4, 8))
                emit_sim(0, s3, ps234)
                emit_s3(1, s3, ps234)
                emit_proj(1, s3, ps234)
                emit_sim(1, s3, ps234)

    nc.compile()
    return nc


def host_fold(probe, wq, wk, bq, wv, bv, wo, bo, ln_g, ln_b, proj_w, proj_b,
              logit_scale, logit_bias):
    """Fold weights on the host into device-friendly layouts."""
    f64 = np.float64
    qvec = probe.reshape(-1).astype(f64) @ wq.astype(f64).T + bq.astype(f64)
    q = qvec.reshape(NH, DH)
    ck = np.stack(
        [q[h] @ wk.astype(f64)[h * DH:(h + 1) * DH, :] for h in range(NH)]
    )  # [16, 1024]
    ck /= np.sqrt(f64(DH))
    # zero-pad heads to M=32 so the scores matmul writes full 32-row blocks;
    # scale into fp8 normal range (values ~1e-3 are subnormal in e4m3)
    ckp = np.zeros((32, DV), np.float64)
    ckp[:NH] = ck * CKSCALE
    # ck8[ci, co, m] = ckp[m, co*128+ci]
    ck8 = np.ascontiguousarray(
        ckp.T.reshape(8, 128, 32).transpose(1, 0, 2)
    ).astype(ml_dtypes.float8_e4m3)

    wvt = np.ascontiguousarray(
        wv.T.reshape(8, 128, DV).transpose(1, 0, 2)).astype(ml_dtypes.bfloat16)
    wot = np.ascontiguousarray(
        wo.T.reshape(8, 128, DV).transpose(1, 0, 2)).astype(ml_dtypes.bfloat16)
    # fold LayerNorm gain/bias into the audio projection:
    #   proj(LN_affine(z)) = z @ (proj_w * g)^T + (proj_b + proj_w @ b)
    pw = proj_w.astype(f64) * ln_g.astype(f64)[None, :]
    pb = proj_b.astype(f64) + proj_w.astype(f64) @ ln_b.astype(f64)
    pjt = np.ascontiguousarray(
        pw.T.reshape(8, 128, DA).transpose(1, 0, 2)).astype(ml_dtypes.bfloat16)
    bo2f = bo.astype(f64) + wo.astype(f64) @ bv.astype(f64)
    sca = np.array([[np.exp(np.float64(logit_scale[0])), logit_bias[0]]],
                   np.float32)
    return dict(
        ck8=ck8, wvt=wvt, wot=wot,
        bo2=bo2f.reshape(1, DV).astype(np.float32),
        pjt=pjt, pjb=pb.reshape(1, DA).astype(np.float32),
        sca=sca,
    )


def host_layouts(video_b, audio_b):
    """Per-core activation layouts (cast + permute only, no FLOPs)."""
    xn = np.ascontiguousarray(
        video_b.reshape(T, 2, 128, DV).transpose(2, 0, 1, 3)
    ).astype(ml_dtypes.float8_e3m4)
    xt8 = np.ascontiguousarray(
        video_b.reshape(T, S, 8, 128).transpose(3, 0, 2, 1)
    ).astype(ml_dtypes.float8_e4m3)
    audt = np.ascontiguousarray(
        audio_b.T.reshape(4, 128, L2).transpose(1, 0, 2)
    ).astype(ml_dtypes.bfloat16)
    return dict(xn=xn, xt8=xt8, audt=audt)


_NC_CACHE = {}


def kernel(video_x, audio_x, probe, wq, wk, wv, bq, bk, bv, wo, bo,
           ln_g, ln_b, proj_w, proj_b, logit_scale, logit_bias, T=None, H=None,
           W=None, **_unused):
    from concourse.bass_utils import run_bass_kernel_spmd

    video_x = np.asarray(video_x, np.float32)
    audio_x = np.asarray(audio_x, np.float32)
    params = host_fold(
        np.asarray(probe, np.float32), np.asarray(wq, np.float32),
        np.asarray(wk, np.float32), np.asarray(bq, np.float32),
        np.asarray(wv, np.float32), np.asarray(bv, np.float32),
        np.asarray(wo, np.float32), np.asarray(bo, np.float32),
        np.asarray(ln_g, np.float32), np.asarray(ln_b, np.float32),
        np.asarray(proj_w, np.float32), np.asarray(proj_b, np.float32),
        np.asarray(logit_scale, np.float32), np.asarray(logit_bias, np.float32),
    )
    if "nc" not in _NC_CACHE:
        _NC_CACHE["nc"] = build_nc()
    nc = _NC_CACHE["nc"]
    in_maps = []
    for b in range(B):
        m = dict(params)
        m.update(host_layouts(video_x[b], audio_x[b]))
        in_maps.append(m)
    res = run_bass_kernel_spmd(nc, in_maps, core_ids=list(range(B)), trace=False)
    return np.stack([res.results[b]["out"] for b in range(B)], axis=0)
